# revision 1
# baseline (speedup 1.0000x reference)
"""Multi-head attention (B=2, S=2048, E=1024, H=16) on 8 TRN2 NeuronCores.

Sharding: batch x head-group. Core c handles batch b=c//4 and head group
g=c%4 (4 heads = 256 of E). Each core computes its heads' attention output
slice and a partial fc_out product [S, E]; the host sums the 4 partials per
batch and adds b_out.

Device-side math per core (all matmuls in float32r, full PE rate):
  qpT = (Wq_g @ q[b].T + bq)      [256, S]   (T layout: dims on partitions)
  kpT = (Wk_g @ k_c[b].T + bk)    [256, SKV] (k compressed by mask, padded)
  vp  = (v_c[b] @ Wv_g.T + bv)*m  [SKV, 4*65] (per head: 64 dims + ones col)
  S_T = kpT_h.T-chunks @ qpT_h    [SKV, S] per head (2 heads row-packed, K=64)
  E_T = exp(S_T)                  (no max-subtraction: |energy| <~ 60, safe)
  AV  = vp_aug.T @ E_T  -> [65, S]: rows 0-63 = unnormalized O_T, row 64 =
        softmax denominator (ones-column trick; pad rows contribute 0)
  O_T = AV[0:64] / AV[64]         (recip + gpsimd partition-broadcast)
  out_partial = O_T.T @ Wo_g.T    [S, E]

Mask handling is exact: masked K/V rows are removed on the host (gather),
so softmax(where(mask==0, -1e20, e)) == exp(e_valid)/sum(exp(e_valid)).
"""

import os

import numpy as np

B, S, E, H = 2, 2048, 1024, 16
D = E // H           # 64
NCORES = 8
GROUPS = 4           # head groups per batch (cores per batch)
HPG = H // GROUPS    # 4 heads per core
DC = E // GROUPS     # 256 dims per core
NB = E // 128        # 8 contraction chunks over E
SQB = 256            # sq block width for scores/AV
NSQB = S // SQB      # 8

_CACHE = {}


def _split_excess_waits(nc, max_waits=1):
    """walrus rejects instructions carrying >1 sem wait; spread extras onto
    single-wait NoOps inserted before the instruction on the same engine."""
    import concourse.mybir as mybir

    n_split = 0
    for f in nc.m.functions:
        for bb in f.blocks:
            out, changed = [], False
            for ins in bb.instructions:
                si = ins.sync_info
                if si is not None and si.on_wait is not None and len(si.on_wait) > max_waits:
                    waits = list(si.on_wait)
                    for w in waits[:-max_waits]:
                        out.append(mybir.InstNoOp(
                            name=nc.get_next_instruction_name(),
                            engine=ins.engine, ins=[], outs=[],
                            sync_info=mybir.SyncInfo(on_wait=[w], on_update=[])))
                        n_split += 1
                    ins.sync_info = mybir.SyncInfo(
                        on_wait=waits[-max_waits:], on_update=list(si.on_update))
                    changed = True
                out.append(ins)
            if changed:
                bb.instructions = out
    return n_split


def _build(skv, split_waits=True):
    import concourse.bass as bass
    import concourse.mybir as mybir
    import concourse.tile as tile

    f32 = mybir.dt.float32
    f32r = mybir.dt.float32r
    f16 = mybir.dt.float16
    bf16 = mybir.dt.bfloat16
    Alu = mybir.AluOpType
    Act = mybir.ActivationFunctionType

    nsk = skv // 128
    kblocks = []
    rem = skv
    while rem > 0:
        w = 384 if rem % 384 == 0 else min(256, rem)
        kblocks.append(w)
        rem -= w

    nc = bass.Bass()
    xqT = nc.declare_dram_parameter("xqT", [E, S], f32r, isOutput=False)
    xkT = nc.declare_dram_parameter("xkT", [E, skv], f32r, isOutput=False)
    xvT = nc.declare_dram_parameter("xvT", [E, skv], f16, isOutput=False)
    wqT = nc.declare_dram_parameter("wqT", [E, DC], f32r, isOutput=False)
    wkT = nc.declare_dram_parameter("wkT", [E, DC], f32r, isOutput=False)
    wvT = nc.declare_dram_parameter("wvT", [E, DC], f16, isOutput=False)
    woT = nc.declare_dram_parameter("woT", [DC, E], f16, isOutput=False)
    bq_d = nc.declare_dram_parameter("bq", [DC], f32, isOutput=False)
    bk_d = nc.declare_dram_parameter("bk", [DC], f32, isOutput=False)
    bv_d = nc.declare_dram_parameter("bv", [DC], f32, isOutput=False)
    vm_d = nc.declare_dram_parameter("vmask", [skv], f32, isOutput=False)
    ones_d = nc.declare_dram_parameter("ones64", [1, 64], f32r, isOutput=False)
    out_d = nc.declare_dram_parameter("out", [2, S, E], f16, isOutput=True)
    srow_d = nc.dram_tensor("srow", [2, 2, S], f32)
    rrow_d = nc.dram_tensor("rrow", [2, 2, S], f32r)

    xqT_r = xqT.rearrange("(ko p) s -> p ko s", p=128)
    xkT_r = xkT.rearrange("(ko p) s -> p ko s", p=128)
    xvT_r = xvT.rearrange("(ko p) s -> p ko s", p=128)

    QB = 512

    with tile.TileContext(nc) as tc:
        with (
            tc.tile_pool(name="weights", bufs=4) as wpool,
            tc.tile_pool(name="consts", bufs=1) as cpool,
            tc.tile_pool(name="persist", bufs=1) as ppool,
            tc.tile_pool(name="small", bufs=2) as smpool,
            tc.tile_pool(name="proj_ps", bufs=1, space="PSUM") as pps,
            tc.tile_pool(name="stream", bufs=2) as spool,
            tc.tile_pool(name="att_ps", bufs=2, space="PSUM") as aps,
            tc.tile_pool(name="av_ps", bufs=1, space="PSUM") as avps,
            tc.tile_pool(name="fc_ps", bufs=2, space="PSUM") as fps,
            tc.tile_pool(name="et", bufs=3) as etpool,
            tc.tile_pool(name="outp", bufs=3) as opool,
            tc.tile_pool(name="sums", bufs=2) as supool,
            tc.tile_pool(name="rcr", bufs=1) as rcpool,
        ):
            # ---- weights / constants (k first: kpT gates attention) ----
            wk_t = wpool.tile([128, NB, DC], f32r, tag="w", name="wk_t")
            wq_t = wpool.tile([128, NB, DC], f32r, tag="w", name="wq_t")
            wv_t = wpool.tile([128, NB, DC], f16, tag="w", name="wv_t")
            wo_t = wpool.tile([128, DC // 128, E], f16, tag="w", name="wo_t")
            nc.sync.dma_start(wk_t[:], wkT.rearrange("(ko p) m -> p ko m", p=128))
            bq_t = cpool.tile([128, 2], f32, tag="bq")
            bk_t = cpool.tile([128, 2], f32, tag="bk")
            bv_t = cpool.tile([128, DC], f32, tag="bv")
            vm_t = cpool.tile([128, nsk], f32, tag="vm")
            nc.sync.dma_start(bk_t[:], bk_d.rearrange("(c p) -> p c", p=128))
            nc.sync.dma_start(bq_t[:], bq_d.rearrange("(c p) -> p c", p=128))
            nc.sync.dma_start(bv_t[:], bv_d[None, :].to_broadcast((128, DC)))
            nc.sync.dma_start(vm_t[:], vm_d.rearrange("(s p) -> p s", p=128))
            ones_t = cpool.tile([1, 64], f32r, tag="ones")
            nc.sync.dma_start(ones_t[:], ones_d[:])

            qpT = ppool.tile([128, 2, S], f32r, tag="qpT")
            kpT = ppool.tile([128, 2, skv], f32r, tag="kpT")
            vp = ppool.tile([128, nsk, HPG * (D + 1)], bf16, tag="vp")
            o_un = ppool.tile([128, 2, S], f32, tag="o_un")
            o_f16 = ppool.tile([128, 2, S], f16, tag="o_f16")

            def proj_k():
                off = 0
                for w in kblocks:
                    xk = spool.tile([128, NB, max(kblocks)], f32r, tag="xk", name="xk")
                    nc.sync.dma_start(xk[:, :, :w], xkT_r[:, :, off:off + w])
                    for mc in range(2):
                        ps = pps.tile([128, 512], f32, tag="pp", name="kp_ps")[:, :max(kblocks)]
                        for kc in range(NB):
                            nc.tensor.matmul(
                                ps[:, :w], wk_t[:, kc, mc * 128:(mc + 1) * 128],
                                xk[:, kc, :w], start=(kc == 0), stop=(kc == NB - 1))
                        nc.vector.tensor_tensor(
                            out=kpT[:, mc, off:off + w], in0=ps[:, :w],
                            in1=bk_t[:, mc:mc + 1].to_broadcast((128, w)), op=Alu.add)
                    off += w

            def proj_q(nb):
                xq = spool.tile([128, NB, 512], f32r, tag="xq", name="xq")
                nc.sync.dma_start(xq[:], xqT_r[:, :, nb * 512:(nb + 1) * 512])
                for mc in range(2):
                    ps = pps.tile([128, 512], f32, tag="pp", name="qp_ps")
                    for kc in range(NB):
                        nc.tensor.matmul(
                            ps[:], wq_t[:, kc, mc * 128:(mc + 1) * 128],
                            xq[:, kc, :], start=(kc == 0), stop=(kc == NB - 1))
                    nc.vector.tensor_tensor(
                        out=qpT[:, mc, nb * 512:(nb + 1) * 512], in0=ps[:],
                        in1=bq_t[:, mc:mc + 1].to_broadcast((128, 512)), op=Alu.add)

            def proj_v(sc):
                xv = spool.tile([128, NB, 128], f16, tag="xv", name="xv")
                nc.sync.dma_start(xv[:], xvT_r[:, :, sc * 128:(sc + 1) * 128])
                ps = pps.tile([128, 512], f32, tag="pp", name="vp_ps")[:, :DC]
                for kc in range(NB):
                    nc.tensor.matmul(
                        ps[:], xv[:, kc, :], wv_t[:, kc, :],
                        start=(kc == 0), stop=(kc == NB - 1))
                t1 = smpool.tile([128, DC], f32, tag="vtmp")
                nc.vector.tensor_tensor(out=t1[:], in0=ps[:], in1=bv_t[:], op=Alu.add)
                vps = vp[:, sc, :].rearrange("p (h w) -> p h w", w=D + 1)
                nc.vector.tensor_tensor(
                    out=vps[:, :, 0:D],
                    in0=t1.rearrange("p (h w) -> p h w", w=D),
                    in1=vm_t[:, sc:sc + 1, None].to_broadcast((128, HPG, D)),
                    op=Alu.mult)
                nc.vector.tensor_copy(
                    out=vps[:, :, D:D + 1],
                    in_=vm_t[:, sc:sc + 1, None].to_broadcast((128, HPG, 1)))

            # lead-in: kpT fully, first qpT block, vp
            proj_k()
            nc.sync.dma_start(wq_t[:], wqT.rearrange("(ko p) m -> p ko m", p=128))
            proj_q(0)
            nc.sync.dma_start(wv_t[:], wvT.rearrange("(ko p) m -> p ko m", p=128))
            for sc in range(nsk):
                proj_v(sc)
            nc.sync.dma_start(wo_t[:], woT.rearrange("(ko p) n -> p ko n", p=128))

            for pt in range(2):
                sums = [supool.tile([1, S], f32, tag="sums", name=f"sums{j}")
                        for j in range(2)]
                for qb in range(S // QB):
                    q0 = qb * QB
                    et = [etpool.tile([128, nsk, QB], bf16, tag="et", name=f"et{j}")
                          for j in range(2)]
                    for skc in range(nsk):
                        psx = [aps.tile([128, QB], f32, tag=f"sc{j}", name=f"psx{j}")
                               for j in range(2)]
                        for j in range(2):
                            nc.tensor.matmul(
                                psx[j][:],
                                kpT[64 * j:64 * j + 64, pt, skc * 128:(skc + 1) * 128],
                                qpT[64 * j:64 * j + 64, pt, q0:q0 + QB],
                                start=True, stop=True, tile_position=(64 * j, 0))
                        for j in range(2):
                            nc.scalar.activation(et[j][:, skc, :], psx[j][:], Act.Exp)
                    for j in range(2):
                        hl = 2 * pt + j
                        ps_av = avps.tile([D + 1, QB], f32, tag="av")
                        for skc in range(nsk):
                            nc.tensor.matmul(
                                ps_av[:], vp[:, skc, hl * (D + 1):(hl + 1) * (D + 1)],
                                et[j][:, skc, :],
                                start=(skc == 0), stop=(skc == nsk - 1))
                        nc.vector.tensor_copy(
                            out=o_un[64 * j:64 * j + 64, pt, q0:q0 + QB],
                            in_=ps_av[0:D, :])
                        nc.vector.tensor_copy(
                            out=sums[j][0:1, q0:q0 + QB], in_=ps_av[D:D + 1, :])
                    # trailing qpT projection blocks interleave with attention
                    # pt=0: full-array matmuls keep the PE dense (and warm).
                    if pt == 0 and qb + 1 < S // QB:
                        proj_q(qb + 1)
                # normalize pair pt
                for j in range(2):
                    s128 = smpool.tile([128, S // 128], f32, tag="s128")
                    nc.sync.dma_start(s128[:], sums[j][0:1, :])
                    nc.vector.reciprocal(out=s128[:], in_=s128[:])
                    r128 = smpool.tile([128, S // 128], f32r, tag="r128")
                    nc.vector.tensor_copy(out=r128[:], in_=s128[:])
                    rc_r = rcpool.tile([1, S], f32r, tag="rcr")
                    nc.sync.dma_start(rc_r[0:1, :], r128[:])
                    for qb in range(S // 512):
                        rc_ps = avps.tile([64, 512], f32, tag="av", name="rc_ps")
                        nc.tensor.matmul(
                            rc_ps[:], ones_t[:], rc_r[0:1, qb * 512:(qb + 1) * 512],
                            start=True, stop=True)
                        nc.vector.tensor_tensor(
                            out=o_f16[64 * j:64 * j + 64, pt, qb * 512:(qb + 1) * 512],
                            in0=o_un[64 * j:64 * j + 64, pt, qb * 512:(qb + 1) * 512],
                            in1=rc_ps[:], op=Alu.mult)
                # fc_out pass for this pair
                for sqc in range(S // 128):
                    for eb in range(2):
                        ps = fps.tile([128, 512], f32, tag="fc")
                        nc.tensor.matmul(
                            ps[:], o_f16[:, pt, sqc * 128:(sqc + 1) * 128],
                            wo_t[:, pt, eb * 512:(eb + 1) * 512],
                            start=True, stop=True)
                        ob = opool.tile([128, 512], f16, tag="ob")
                        nc.any.tensor_copy(out=ob[:], in_=ps[:])
                        nc.sync.dma_start(
                            out_d[pt, sqc * 128:(sqc + 1) * 128,
                                  eb * 512:(eb + 1) * 512],
                            ob[:])

    if split_waits:
        _split_excess_waits(nc)
    return nc


def _prep_inputs(q, k, v, mask, W_qkv, b_qkv, W_out, b_out):
    """Host-side shard/layout prep. Returns (skv, in_maps)."""
    q = np.asarray(q, dtype=np.float32)
    k = np.asarray(k, dtype=np.float32)
    v = np.asarray(v, dtype=np.float32)
    mask = np.asarray(mask)
    W_qkv = np.asarray(W_qkv, dtype=np.float32)
    b_qkv = np.asarray(b_qkv, dtype=np.float32)
    W_out = np.asarray(W_out, dtype=np.float32)

    valid = [np.nonzero(mask[b, 0, 0] != 0)[0] for b in range(B)]
    cnts = [len(vi) for vi in valid]
    skv = max(128, max((c + 127) // 128 * 128 for c in cnts))

    # per-batch tensors
    qT, kTc, vTc, vms = [], [], [], []
    for b in range(B):
        qT.append(np.ascontiguousarray(q[b].T))
        kt = np.zeros((E, skv), np.float32)
        vt = np.zeros((E, skv), np.float16)
        kt[:, :cnts[b]] = k[b][valid[b]].T
        vt[:, :cnts[b]] = v[b][valid[b]].T
        kTc.append(kt)
        vTc.append(vt)
        vm = np.zeros((skv,), np.float32)
        vm[:cnts[b]] = 1.0
        vms.append(vm)

    in_maps = []
    for c in range(NCORES):
        b, g = divmod(c, GROUPS)
        sl = slice(g * DC, (g + 1) * DC)
        in_maps.append({
            "xqT": qT[b], "xkT": kTc[b], "xvT": vTc[b],
            "wqT": np.ascontiguousarray(W_qkv[sl, :].T),
            "wkT": np.ascontiguousarray(W_qkv[E:][sl, :].T),
            "wvT": np.ascontiguousarray(W_qkv[2 * E:][sl, :].T).astype(np.float16),
            "woT": np.ascontiguousarray(W_out[:, sl].T).astype(np.float16),
            "bq": np.ascontiguousarray(b_qkv[sl]),
            "bk": np.ascontiguousarray(b_qkv[E:][sl]),
            "bv": np.ascontiguousarray(b_qkv[2 * E:][sl]),
            "vmask": vms[b],
            "ones64": np.ones((1, 64), np.float32),
        })
    return skv, in_maps


def kernel(q, k, v, mask, W_qkv, b_qkv, W_out, b_out):
    from concourse import bass_utils

    skv, in_maps = _prep_inputs(q, k, v, mask, W_qkv, b_qkv, W_out, b_out)
    if skv not in _CACHE:
        _CACHE[skv] = _build(skv)
    nc = _CACHE[skv]

    trace = os.environ.get("KERNEL_TRACE") == "1"
    if trace:
        bass_utils.upload_artifacts = lambda tmpdir: "local://" + tmpdir
    res = bass_utils.run_bass_kernel_spmd(
        nc, in_maps, list(range(NCORES)), trace=trace)
    if trace:
        print(f"HW exec time: {res.exec_time_ns} ns")

    b_out = np.asarray(b_out, dtype=np.float32)
    out = np.zeros((B, S, E), np.float32)
    for c in range(NCORES):
        out[c // GROUPS] += res.results[c]["out"].astype(np.float32).sum(axis=0)
    out += b_out[None, None, :]
    return out



# revision 2
# speedup vs baseline: 1.3218x; 1.3218x over previous
"""Multi-head attention (B=2, S=2048, E=1024, H=16) on 8 TRN2 NeuronCores.

Sharding: batch x head-group. Core c handles batch b=c//4 and head group
g=c%4 (4 heads = 256 of E). Each core computes its heads' attention output
slice and a partial fc_out product [S, E]; the host sums the 4 partials per
batch and adds b_out.

Device-side math per core (all matmuls in float32r, full PE rate):
  qpT = (Wq_g @ q[b].T + bq)      [256, S]   (T layout: dims on partitions)
  kpT = (Wk_g @ k_c[b].T + bk)    [256, SKV] (k compressed by mask, padded)
  vp  = (v_c[b] @ Wv_g.T + bv)*m  [SKV, 4*65] (per head: 64 dims + ones col)
  S_T = kpT_h.T-chunks @ qpT_h    [SKV, S] per head (2 heads row-packed, K=64)
  E_T = exp(S_T)                  (no max-subtraction: |energy| <~ 60, safe)
  AV  = vp_aug.T @ E_T  -> [65, S]: rows 0-63 = unnormalized O_T, row 64 =
        softmax denominator (ones-column trick; pad rows contribute 0)
  O_T = AV[0:64] / AV[64]         (recip + PE partition-broadcast)
  out_partial = O_T.T @ Wo_g.T    [S, E] (both head-pairs accumulated in PSUM)

Mask handling is exact: masked K/V rows are removed on the host (gather),
so softmax(where(mask==0, -1e20, e)) == exp(e_valid)/sum(exp(e_valid)).

Pipeline: the attention loop is software-pipelined over 512-wide query
blocks: block s emits [AV of block s-1 | scores of block s | exp of block s]
interleaved per skv-chunk, so the Scalar engine (exp, the per-block
bottleneck) always has scores available and the PE back-fills with AV,
Q-projection (pt=0) or fc_out (pt=1) work. exp runs as one 1024-wide
instruction per skv-chunk over both row-packed heads (2 adjacent PSUM
banks) to amortize ACT fixed overheads. Softmax normalization is per-block
so fc_out can start before the whole pair finishes.
"""

import os

import numpy as np

B, S, E, H = 2, 2048, 1024, 16
D = E // H           # 64
NCORES = 8
GROUPS = 4           # head groups per batch (cores per batch)
HPG = H // GROUPS    # 4 heads per core
DC = E // GROUPS     # 256 dims per core
NB = E // 128        # 8 contraction chunks over E
QB = 512             # query block width
NQB = S // QB        # 4

_CACHE = {}


def _split_excess_waits(nc, max_waits=1):
    """walrus rejects instructions carrying >1 sem wait; spread extras onto
    single-wait NoOps inserted before the instruction on the same engine."""
    import concourse.mybir as mybir

    n_split = 0
    for f in nc.m.functions:
        for bb in f.blocks:
            out, changed = [], False
            for ins in bb.instructions:
                si = ins.sync_info
                if si is not None and si.on_wait is not None and len(si.on_wait) > max_waits:
                    waits = list(si.on_wait)
                    for w in waits[:-max_waits]:
                        out.append(mybir.InstNoOp(
                            name=nc.get_next_instruction_name(),
                            engine=ins.engine, ins=[], outs=[],
                            sync_info=mybir.SyncInfo(on_wait=[w], on_update=[])))
                        n_split += 1
                    ins.sync_info = mybir.SyncInfo(
                        on_wait=waits[-max_waits:], on_update=list(si.on_update))
                    changed = True
                out.append(ins)
            if changed:
                bb.instructions = out
    return n_split


def _build(skv, split_waits=True):
    import concourse.bass as bass
    import concourse.mybir as mybir
    import concourse.tile as tile

    f32 = mybir.dt.float32
    f32r = mybir.dt.float32r
    f16 = mybir.dt.float16
    bf16 = mybir.dt.bfloat16
    Alu = mybir.AluOpType
    Act = mybir.ActivationFunctionType

    nsk = skv // 128
    kblocks = []
    rem = skv
    while rem > 0:
        w = 384 if rem % 384 == 0 else min(256, rem)
        kblocks.append(w)
        rem -= w

    nc = bass.Bass()
    xqT = nc.declare_dram_parameter("xqT", [E, S], f32r, isOutput=False)
    xkT = nc.declare_dram_parameter("xkT", [E, skv], f32r, isOutput=False)
    xvT = nc.declare_dram_parameter("xvT", [E, skv], f16, isOutput=False)
    wqT = nc.declare_dram_parameter("wqT", [E, DC], f32r, isOutput=False)
    wkT = nc.declare_dram_parameter("wkT", [E, DC], f32r, isOutput=False)
    wvT = nc.declare_dram_parameter("wvT", [E, DC], f16, isOutput=False)
    woT = nc.declare_dram_parameter("woT", [DC, E], f16, isOutput=False)
    bq_d = nc.declare_dram_parameter("bq", [DC], f32, isOutput=False)
    bk_d = nc.declare_dram_parameter("bk", [DC], f32, isOutput=False)
    bv_d = nc.declare_dram_parameter("bv", [DC], f32, isOutput=False)
    vm_d = nc.declare_dram_parameter("vmask", [skv], f32, isOutput=False)
    ones_d = nc.declare_dram_parameter("ones64", [1, 64], f32r, isOutput=False)
    out_d = nc.declare_dram_parameter("out", [S, E], f16, isOutput=True)

    xqT_r = xqT.rearrange("(ko p) s -> p ko s", p=128)
    xkT_r = xkT.rearrange("(ko p) s -> p ko s", p=128)
    xvT_r = xvT.rearrange("(ko p) s -> p ko s", p=128)

    with tile.TileContext(nc) as tc:
        with (
            tc.tile_pool(name="weights", bufs=1) as wpool,
            tc.tile_pool(name="consts", bufs=1) as cpool,
            tc.tile_pool(name="persist", bufs=1) as ppool,
            tc.tile_pool(name="small", bufs=2) as smpool,
            tc.tile_pool(name="stream", bufs=2) as spool,
            tc.tile_pool(name="px_ps", bufs=2, space="PSUM") as pxps,
            tc.tile_pool(name="av_ps", bufs=2, space="PSUM") as avps,
            tc.tile_pool(name="gen_ps", bufs=2, space="PSUM") as gps,
            tc.tile_pool(name="et", bufs=2) as etpool,
            tc.tile_pool(name="outp", bufs=3) as opool,
            tc.tile_pool(name="rcr", bufs=2) as rcpool,
        ):
            # ---- weights / constants (k first: kpT gates attention) ----
            wk_t = wpool.tile([128, NB, DC], f32r, tag="wk", name="wk_t")
            wq_t = wpool.tile([128, NB, DC], f32r, tag="wq", name="wq_t")
            wv_t = wpool.tile([128, NB, DC], f16, tag="wv", name="wv_t")
            wo_t = wpool.tile([128, DC // 128, E], f16, tag="wo", name="wo_t")
            nc.sync.dma_start(wk_t[:], wkT.rearrange("(ko p) m -> p ko m", p=128))
            bq_t = cpool.tile([128, 2], f32, tag="bq")
            bk_t = cpool.tile([128, 2], f32, tag="bk")
            bv_t = cpool.tile([128, DC], f32, tag="bv")
            vm_t = cpool.tile([128, nsk], f32, tag="vm")
            nc.sync.dma_start(bk_t[:], bk_d.rearrange("(c p) -> p c", p=128))
            nc.sync.dma_start(bq_t[:], bq_d.rearrange("(c p) -> p c", p=128))
            nc.sync.dma_start(bv_t[:], bv_d[None, :].to_broadcast((128, DC)))
            nc.sync.dma_start(vm_t[:], vm_d.rearrange("(s p) -> p s", p=128))
            ones_t = cpool.tile([1, 64], f32r, tag="ones")
            nc.sync.dma_start(ones_t[:], ones_d[:])

            qpT = ppool.tile([128, 2, S], f32r, tag="qpT")
            kpT = ppool.tile([128, 2, skv], f32r, tag="kpT")
            vp = ppool.tile([128, nsk, HPG * (D + 1)], bf16, tag="vp")
            # per-j unnormalized O_T (rows 0-63) + denominator (row 64)
            o_un0 = ppool.tile([65, 2, S], f32, tag="o_un0")
            o_un1 = ppool.tile([65, 2, S], f32, tag="o_un1")
            o_uns = [o_un0, o_un1]
            o_f16 = ppool.tile([128, 2, S], f16, tag="o_f16")

            def proj_k():
                off = 0
                for w in kblocks:
                    xk = spool.tile([128, NB, max(kblocks)], f32r, tag="xk", name="xk")
                    nc.sync.dma_start(xk[:, :, :w], xkT_r[:, :, off:off + w])
                    for mc in range(2):
                        ps = gps.tile([128, 512], f32, tag="gp", name="kp_ps")[:, :max(kblocks)]
                        for kc in range(NB):
                            nc.tensor.matmul(
                                ps[:, :w], wk_t[:, kc, mc * 128:(mc + 1) * 128],
                                xk[:, kc, :w], start=(kc == 0), stop=(kc == NB - 1))
                        nc.vector.tensor_tensor(
                            out=kpT[:, mc, off:off + w], in0=ps[:, :w],
                            in1=bk_t[:, mc:mc + 1].to_broadcast((128, w)), op=Alu.add)
                    off += w

            def proj_q(nb):
                xq = spool.tile([128, NB, 512], f32r, tag="xq", name="xq")
                nc.sync.dma_start(xq[:], xqT_r[:, :, nb * 512:(nb + 1) * 512])
                for mc in range(2):
                    ps = gps.tile([128, 512], f32, tag="gp", name="qp_ps")
                    for kc in range(NB):
                        nc.tensor.matmul(
                            ps[:], wq_t[:, kc, mc * 128:(mc + 1) * 128],
                            xq[:, kc, :], start=(kc == 0), stop=(kc == NB - 1))
                    nc.vector.tensor_tensor(
                        out=qpT[:, mc, nb * 512:(nb + 1) * 512], in0=ps[:],
                        in1=bq_t[:, mc:mc + 1].to_broadcast((128, 512)), op=Alu.add)

            def proj_v(sc):
                xv = spool.tile([128, NB, 128], f16, tag="xv", name="xv")
                nc.sync.dma_start(xv[:], xvT_r[:, :, sc * 128:(sc + 1) * 128])
                ps = gps.tile([128, 512], f32, tag="gp", name="vp_ps")[:, :DC]
                for kc in range(NB):
                    nc.tensor.matmul(
                        ps[:], xv[:, kc, :], wv_t[:, kc, :],
                        start=(kc == 0), stop=(kc == NB - 1))
                t1 = smpool.tile([128, DC], f32, tag="vtmp")
                nc.vector.tensor_tensor(out=t1[:], in0=ps[:], in1=bv_t[:], op=Alu.add)
                vps = vp[:, sc, :].rearrange("p (h w) -> p h w", w=D + 1)
                nc.vector.tensor_tensor(
                    out=vps[:, :, 0:D],
                    in0=t1.rearrange("p (h w) -> p h w", w=D),
                    in1=vm_t[:, sc:sc + 1, None].to_broadcast((128, HPG, D)),
                    op=Alu.mult)
                nc.vector.tensor_copy(
                    out=vps[:, :, D:D + 1],
                    in_=vm_t[:, sc:sc + 1, None].to_broadcast((128, HPG, 1)))

            def finish_block(pt, qb, psavs):
                """AV copy + per-block softmax normalize for query block qb
                of pair pt (runs one pipeline step after its AV matmuls)."""
                q0 = qb * QB
                for j in range(2):
                    nc.vector.tensor_copy(
                        out=o_uns[j][0:D + 1, pt, q0:q0 + QB],
                        in_=psavs[j][0:D + 1, :])
                for j in range(2):
                    s128 = smpool.tile([128, QB // 128], f32, tag="s128")
                    nc.sync.dma_start(s128[:], o_uns[j][D:D + 1, pt, q0:q0 + QB])
                    nc.vector.reciprocal(out=s128[:], in_=s128[:])
                    r128 = smpool.tile([128, QB // 128], f32r, tag="r128")
                    nc.vector.tensor_copy(out=r128[:], in_=s128[:])
                    rc_r = rcpool.tile([1, QB], f32r, tag="rcr")
                    nc.sync.dma_start(rc_r[0:1, :], r128[:])
                    rc_ps = gps.tile([128, 512], f32, tag="gp", name="rc_ps")
                    nc.tensor.matmul(
                        rc_ps[0:D, :], ones_t[:], rc_r[0:1, :],
                        start=True, stop=True)
                    nc.vector.tensor_tensor(
                        out=o_f16[64 * j:64 * j + 64, pt, q0:q0 + QB],
                        in0=o_uns[j][0:D, pt, q0:q0 + QB],
                        in1=rc_ps[0:D, :], op=Alu.mult)

            def fc_block(b):
                """fc_out for query block b (both pairs accumulated in PSUM)."""
                for sqc in range(b * (QB // 128), (b + 1) * (QB // 128)):
                    for eb in range(2):
                        ps = gps.tile([128, 512], f32, tag="gp", name="fc_ps")
                        nc.tensor.matmul(
                            ps[:], o_f16[:, 0, sqc * 128:(sqc + 1) * 128],
                            wo_t[:, 0, eb * 512:(eb + 1) * 512],
                            start=True, stop=False)
                        nc.tensor.matmul(
                            ps[:], o_f16[:, 1, sqc * 128:(sqc + 1) * 128],
                            wo_t[:, 1, eb * 512:(eb + 1) * 512],
                            start=False, stop=True)
                        ob = opool.tile([128, 512], f16, tag="ob")
                        nc.any.tensor_copy(out=ob[:], in_=ps[:])
                        nc.sync.dma_start(
                            out_d[sqc * 128:(sqc + 1) * 128,
                                  eb * 512:(eb + 1) * 512],
                            ob[:])

            # lead-in: kpT fully, first qpT block, vp
            proj_k()
            nc.sync.dma_start(wq_t[:], wqT.rearrange("(ko p) m -> p ko m", p=128))
            proj_q(0)
            nc.sync.dma_start(wv_t[:], wvT.rearrange("(ko p) m -> p ko m", p=128))
            for sc in range(nsk):
                proj_v(sc)
            nc.sync.dma_start(wo_t[:], woT.rearrange("(ko p) n -> p ko n", p=128))

            # software-pipelined attention over 8 (pt, qb) steps
            state = None  # (pt, qb, et tile) awaiting AV
            for pt in range(2):
                for qb in range(NQB):
                    q0 = qb * QB
                    et_t = etpool.tile([128, nsk, 2, QB], bf16, tag="et",
                                       name="et_t")
                    psavs = None
                    if state is not None:
                        psavs = [avps.tile([128, QB], f32, tag="av",
                                           name=f"psav{j}") for j in range(2)]
                    for skc in range(nsk):
                        if state is not None:
                            ppt, pqb, pet = state
                            for j in range(2):
                                hl = 2 * ppt + j
                                nc.tensor.matmul(
                                    psavs[j][0:D + 1, :],
                                    vp[:, skc, hl * (D + 1):(hl + 1) * (D + 1)],
                                    pet[:, skc, j, :],
                                    start=(skc == 0), stop=(skc == nsk - 1))
                        psx = pxps.tile([128, 2, QB], f32, tag="px", name="psx")
                        for j in range(2):
                            nc.tensor.matmul(
                                psx[:, j, :],
                                kpT[64 * j:64 * j + 64, pt, skc * 128:(skc + 1) * 128],
                                qpT[64 * j:64 * j + 64, pt, q0:q0 + QB],
                                start=True, stop=True, tile_position=(64 * j, 0))
                        nc.scalar.activation(et_t[:, skc, :, :], psx[:], Act.Exp)
                    if state is not None:
                        finish_block(state[0], state[1], psavs)
                    state = (pt, qb, et_t)
                    # PE filler during the ACT-bound exp phase
                    if pt == 0 and qb + 1 < NQB:
                        proj_q(qb + 1)
                    elif pt == 1 and qb >= 2:
                        fc_block(qb - 2)
            # drain: AV + normalize of the last block, then remaining fc
            psavs = [avps.tile([128, QB], f32, tag="av", name=f"psavd{j}")
                     for j in range(2)]
            for skc in range(nsk):
                ppt, pqb, pet = state
                for j in range(2):
                    hl = 2 * ppt + j
                    nc.tensor.matmul(
                        psavs[j][0:D + 1, :],
                        vp[:, skc, hl * (D + 1):(hl + 1) * (D + 1)],
                        pet[:, skc, j, :],
                        start=(skc == 0), stop=(skc == nsk - 1))
            finish_block(state[0], state[1], psavs)
            fc_block(2)
            fc_block(3)

    if split_waits:
        _split_excess_waits(nc)
    return nc


def _prep_inputs(q, k, v, mask, W_qkv, b_qkv, W_out, b_out):
    """Host-side shard/layout prep. Returns (skv, in_maps)."""
    q = np.asarray(q, dtype=np.float32)
    k = np.asarray(k, dtype=np.float32)
    v = np.asarray(v, dtype=np.float32)
    mask = np.asarray(mask)
    W_qkv = np.asarray(W_qkv, dtype=np.float32)
    b_qkv = np.asarray(b_qkv, dtype=np.float32)
    W_out = np.asarray(W_out, dtype=np.float32)

    valid = [np.nonzero(mask[b, 0, 0] != 0)[0] for b in range(B)]
    cnts = [len(vi) for vi in valid]
    skv = max(128, max((c + 127) // 128 * 128 for c in cnts))

    # per-batch tensors
    qT, kTc, vTc, vms = [], [], [], []
    for b in range(B):
        qT.append(np.ascontiguousarray(q[b].T))
        kt = np.zeros((E, skv), np.float32)
        vt = np.zeros((E, skv), np.float16)
        kt[:, :cnts[b]] = k[b][valid[b]].T
        vt[:, :cnts[b]] = v[b][valid[b]].T
        kTc.append(kt)
        vTc.append(vt)
        vm = np.zeros((skv,), np.float32)
        vm[:cnts[b]] = 1.0
        vms.append(vm)

    in_maps = []
    for c in range(NCORES):
        b, g = divmod(c, GROUPS)
        sl = slice(g * DC, (g + 1) * DC)
        in_maps.append({
            "xqT": qT[b], "xkT": kTc[b], "xvT": vTc[b],
            "wqT": np.ascontiguousarray(W_qkv[sl, :].T),
            "wkT": np.ascontiguousarray(W_qkv[E:][sl, :].T),
            "wvT": np.ascontiguousarray(W_qkv[2 * E:][sl, :].T).astype(np.float16),
            "woT": np.ascontiguousarray(W_out[:, sl].T).astype(np.float16),
            "bq": np.ascontiguousarray(b_qkv[sl]),
            "bk": np.ascontiguousarray(b_qkv[E:][sl]),
            "bv": np.ascontiguousarray(b_qkv[2 * E:][sl]),
            "vmask": vms[b],
            "ones64": np.ones((1, 64), np.float32),
        })
    return skv, in_maps


def kernel(q, k, v, mask, W_qkv, b_qkv, W_out, b_out):
    from concourse import bass_utils

    skv, in_maps = _prep_inputs(q, k, v, mask, W_qkv, b_qkv, W_out, b_out)
    if skv not in _CACHE:
        _CACHE[skv] = _build(skv)
    nc = _CACHE[skv]

    trace = os.environ.get("KERNEL_TRACE") == "1"
    if trace:
        bass_utils.upload_artifacts = lambda tmpdir: "local://" + tmpdir
    res = bass_utils.run_bass_kernel_spmd(
        nc, in_maps, list(range(NCORES)), trace=trace)
    if trace:
        print(f"HW exec time: {res.exec_time_ns} ns")

    b_out = np.asarray(b_out, dtype=np.float32)
    out = np.zeros((B, S, E), np.float32)
    for c in range(NCORES):
        out[c // GROUPS] += res.results[c]["out"].astype(np.float32)
    out += b_out[None, None, :]
    return out


# revision 12
# speedup vs baseline: 1.4285x; 1.0807x over previous
"""Multi-head attention (B=2, S=2048, E=1024, H=16) on 8 TRN2 NeuronCores.

Sharding: batch x head-group. Core c handles batch b=c//4 and head group
g=c%4 (4 heads = 256 of E). Each core computes its heads' attention output
slice and a partial fc_out product [S, E]; the host sums the 4 partials per
batch and adds b_out.

Device-side math per core (all matmuls in float32r, full PE rate):
  qpT = (Wq_g @ q[b].T + bq)      [256, S]   (T layout: dims on partitions)
  kpT = (Wk_g @ k_c[b].T + bk)    [256, SKV] (k compressed by mask, padded)
  vp  = (v_c[b] @ Wv_g.T + bv)*m  [SKV, 4*65] (per head: 64 dims + ones col)
  S_T = kpT_h.T-chunks @ qpT_h    [SKV, S] per head (2 heads row-packed, K=64)
  E_T = exp(S_T)                  (no max-subtraction: |energy| <~ 60, safe)
  AV  = vp_aug.T @ E_T  -> [65, S]: rows 0-63 = unnormalized O_T, row 64 =
        softmax denominator (ones-column trick; pad rows contribute 0)
  O_T = AV[0:64] / AV[64]         (recip + PE partition-broadcast)
  out_partial = O_T.T @ Wo_g.T    [S, E] (both head-pairs accumulated in PSUM)

Mask handling is exact: masked K/V rows are removed on the host (gather),
so softmax(where(mask==0, -1e20, e)) == exp(e_valid)/sum(exp(e_valid)).

Pipeline: the attention loop is software-pipelined over 512-wide query
blocks: block s emits [AV of block s-1 | scores of block s | exp of block s]
interleaved per skv-chunk, so the Scalar engine (exp, the per-block
bottleneck) always has scores available and the PE back-fills with AV,
Q-projection (pt=0) or fc_out (pt=1) work. exp runs as one 1024-wide
instruction per skv-chunk over both row-packed heads (2 adjacent PSUM
banks) to amortize ACT fixed overheads. Softmax normalization is per-block
so fc_out can start before the whole pair finishes.
"""

import os

import numpy as np

B, S, E, H = 2, 2048, 1024, 16
D = E // H           # 64
NCORES = 8
GROUPS = 4           # head groups per batch (cores per batch)
HPG = H // GROUPS    # 4 heads per core
DC = E // GROUPS     # 256 dims per core
NB = E // 128        # 8 contraction chunks over E
QB = 512             # query block width
NQB = S // QB        # 4

_CACHE = {}


def _split_excess_waits(nc, max_waits=1):
    """walrus rejects instructions carrying >1 sem wait; spread extras onto
    single-wait NoOps inserted before the instruction on the same engine."""
    import concourse.mybir as mybir

    n_split = 0
    for f in nc.m.functions:
        for bb in f.blocks:
            out, changed = [], False
            for ins in bb.instructions:
                si = ins.sync_info
                if si is not None and si.on_wait is not None and len(si.on_wait) > max_waits:
                    waits = list(si.on_wait)
                    for w in waits[:-max_waits]:
                        out.append(mybir.InstNoOp(
                            name=nc.get_next_instruction_name(),
                            engine=ins.engine, ins=[], outs=[],
                            sync_info=mybir.SyncInfo(on_wait=[w], on_update=[])))
                        n_split += 1
                    ins.sync_info = mybir.SyncInfo(
                        on_wait=waits[-max_waits:], on_update=list(si.on_update))
                    changed = True
                out.append(ins)
            if changed:
                bb.instructions = out
    return n_split


def _build(skv, split_waits=True):
    import concourse.bass as bass
    import concourse.mybir as mybir
    import concourse.tile as tile

    f32 = mybir.dt.float32
    f32r = mybir.dt.float32r
    f16 = mybir.dt.float16
    bf16 = mybir.dt.bfloat16
    Alu = mybir.AluOpType
    Act = mybir.ActivationFunctionType

    nsk = skv // 128
    kblocks = []
    rem = skv
    while rem > 0:
        w = 384 if rem % 384 == 0 else min(256, rem)
        kblocks.append(w)
        rem -= w

    nc = bass.Bass()
    xqT = nc.declare_dram_parameter("xqT", [E, S], f16, isOutput=False)
    xkT = nc.declare_dram_parameter("xkT", [E, skv], f16, isOutput=False)
    xvT = nc.declare_dram_parameter("xvT", [E, skv], f16, isOutput=False)
    wqT = nc.declare_dram_parameter("wqT", [E, DC], f16, isOutput=False)
    wkT = nc.declare_dram_parameter("wkT", [E, DC], f16, isOutput=False)
    wvT = nc.declare_dram_parameter("wvT", [E, DC], f16, isOutput=False)
    woT = nc.declare_dram_parameter("woT", [DC, E], f16, isOutput=False)
    bq_d = nc.declare_dram_parameter("bq", [DC], f32, isOutput=False)
    bk_d = nc.declare_dram_parameter("bk", [DC], f32, isOutput=False)
    bv_d = nc.declare_dram_parameter("bv", [DC], f32, isOutput=False)
    vm_d = nc.declare_dram_parameter("vmask", [skv], f32, isOutput=False)
    sel2_d = nc.declare_dram_parameter("sel2", [2, 128], f32r, isOutput=False)
    out_d = nc.declare_dram_parameter("out", [S, E], f16, isOutput=True)

    xqT_r = xqT.rearrange("(ko p) s -> p ko s", p=128)
    xkT_r = xkT.rearrange("(ko p) s -> p ko s", p=128)
    xvT_r = xvT.rearrange("(ko p) s -> p ko s", p=128)

    with tile.TileContext(nc) as tc:
        with (
            tc.tile_pool(name="weights", bufs=1) as wpool,
            tc.tile_pool(name="consts", bufs=1) as cpool,
            tc.tile_pool(name="persist", bufs=1) as ppool,
            tc.tile_pool(name="small", bufs=2) as smpool,
            tc.tile_pool(name="stream", bufs=2) as spool,
            tc.tile_pool(name="px_ps", bufs=2, space="PSUM") as pxps,
            tc.tile_pool(name="av_ps", bufs=2, space="PSUM") as avps,
            tc.tile_pool(name="gen_ps", bufs=2, space="PSUM") as gps,
            tc.tile_pool(name="et", bufs=2) as etpool,
            tc.tile_pool(name="outp", bufs=3) as opool,
            tc.tile_pool(name="rcr", bufs=2) as rcpool,
        ):
            # ---- weights / constants (k first: kpT gates attention) ----
            wk_t = wpool.tile([128, NB, DC], f16, tag="wk", name="wk_t")
            wq_t = wpool.tile([128, NB, DC], f16, tag="wq", name="wq_t")
            wv_t = wpool.tile([128, NB, DC], f16, tag="wv", name="wv_t")
            wo_t = wpool.tile([128, DC // 128, E], f16, tag="wo", name="wo_t")
            nc.sync.dma_start(wk_t[:], wkT.rearrange("(ko p) m -> p ko m", p=128))
            bq_t = cpool.tile([128, 2], f32, tag="bq")
            bk_t = cpool.tile([128, 2], f32, tag="bk")
            bv_t = cpool.tile([128, DC], f32, tag="bv")
            vm_t = cpool.tile([128, nsk], f32, tag="vm")
            nc.sync.dma_start(bk_t[:], bk_d.rearrange("(c p) -> p c", p=128))
            nc.sync.dma_start(bq_t[:], bq_d.rearrange("(c p) -> p c", p=128))
            nc.sync.dma_start(bv_t[:], bv_d[None, :].to_broadcast((128, DC)))
            nc.sync.dma_start(vm_t[:], vm_d.rearrange("(s p) -> p s", p=128))
            sel2_t = cpool.tile([2, 128], f32r, tag="sel2")
            nc.sync.dma_start(sel2_t[:], sel2_d[:])

            qpT = ppool.tile([128, 2, S], f32r, tag="qpT")
            kpT = ppool.tile([128, 2, skv], f32r, tag="kpT")
            vp = ppool.tile([128, nsk, HPG * (D + 1)], bf16, tag="vp")
            # per-j unnormalized O_T (rows 0-63) + denominator (row 64)
            o_un0 = ppool.tile([65, 2, S], f32, tag="o_un0")
            o_un1 = ppool.tile([65, 2, S], f32, tag="o_un1")
            o_uns = [o_un0, o_un1]
            o_f16 = ppool.tile([128, 2, S], f16, tag="o_f16")

            def proj_k():
                off = 0
                for w in kblocks:
                    xk = spool.tile([128, NB, max(kblocks)], f16, tag="xk", name="xk")
                    nc.sync.dma_start(xk[:, :, :w], xkT_r[:, :, off:off + w])
                    for mc in range(2):
                        ps = gps.tile([128, 512], f32, tag="gp", name="kp_ps")[:, :max(kblocks)]
                        for kc in range(NB):
                            nc.tensor.matmul(
                                ps[:, :w], wk_t[:, kc, mc * 128:(mc + 1) * 128],
                                xk[:, kc, :w], start=(kc == 0), stop=(kc == NB - 1))
                        nc.vector.tensor_tensor(
                            out=kpT[:, mc, off:off + w], in0=ps[:, :w],
                            in1=bk_t[:, mc:mc + 1].to_broadcast((128, w)), op=Alu.add)
                    off += w

            def proj_q(nb):
                xq = spool.tile([128, NB, 512], f16, tag="xq", name="xq")
                nc.sync.dma_start(xq[:], xqT_r[:, :, nb * 512:(nb + 1) * 512])
                for mc in range(2):
                    ps = gps.tile([128, 512], f32, tag="gp", name="qp_ps")
                    for kc in range(NB):
                        nc.tensor.matmul(
                            ps[:], wq_t[:, kc, mc * 128:(mc + 1) * 128],
                            xq[:, kc, :], start=(kc == 0), stop=(kc == NB - 1))
                    nc.vector.tensor_tensor(
                        out=qpT[:, mc, nb * 512:(nb + 1) * 512], in0=ps[:],
                        in1=bq_t[:, mc:mc + 1].to_broadcast((128, 512)), op=Alu.add)

            def proj_v(sc):
                xv = spool.tile([128, NB, 128], f16, tag="xv", name="xv")
                nc.sync.dma_start(xv[:], xvT_r[:, :, sc * 128:(sc + 1) * 128])
                ps = gps.tile([128, 512], f32, tag="gp", name="vp_ps")[:, :DC]
                for kc in range(NB):
                    nc.tensor.matmul(
                        ps[:], xv[:, kc, :], wv_t[:, kc, :],
                        start=(kc == 0), stop=(kc == NB - 1))
                t1 = smpool.tile([128, DC], f32, tag="vtmp")
                nc.vector.tensor_tensor(out=t1[:], in0=ps[:], in1=bv_t[:], op=Alu.add)
                vps = vp[:, sc, :].rearrange("p (h w) -> p h w", w=D + 1)
                nc.vector.tensor_tensor(
                    out=vps[:, :, 0:D],
                    in0=t1.rearrange("p (h w) -> p h w", w=D),
                    in1=vm_t[:, sc:sc + 1, None].to_broadcast((128, HPG, D)),
                    op=Alu.mult)
                nc.vector.tensor_copy(
                    out=vps[:, :, D:D + 1],
                    in_=vm_t[:, sc:sc + 1, None].to_broadcast((128, HPG, 1)))

            def finish_block(pt, qb, psavs):
                """AV copy + per-block softmax normalize for query block qb
                of pair pt (runs one pipeline step after its AV matmuls)."""
                q0 = qb * QB
                for j in range(2):
                    nc.vector.tensor_copy(
                        out=o_uns[j][0:D + 1, pt, q0:q0 + QB],
                        in_=psavs[j][0:D + 1, :])
                rc_r = rcpool.tile([2, QB], f32r, tag="rcr")
                for j in range(2):
                    s128 = smpool.tile([128, QB // 128], f32, tag="s128")
                    nc.sync.dma_start(s128[:], o_uns[j][D:D + 1, pt, q0:q0 + QB])
                    nc.vector.reciprocal(out=s128[:], in_=s128[:])
                    r128 = smpool.tile([128, QB // 128], f32r, tag="r128")
                    nc.vector.tensor_copy(out=r128[:], in_=s128[:])
                    nc.sync.dma_start(rc_r[j:j + 1, :], r128[:])
                # one PE broadcast for both heads: sel2 routes row j of rc_r
                # to partitions 64j..64j+63
                rc_ps = gps.tile([128, 512], f32, tag="gp", name="rc_ps")
                nc.tensor.matmul(
                    rc_ps[:], sel2_t[:], rc_r[:], start=True, stop=True)
                for j in range(2):
                    nc.vector.tensor_tensor(
                        out=o_f16[64 * j:64 * j + 64, pt, q0:q0 + QB],
                        in0=o_uns[j][0:D, pt, q0:q0 + QB],
                        in1=rc_ps[64 * j:64 * j + 64, :], op=Alu.mult)

            def fc_block(b):
                """fc_out for query block b (both pairs accumulated in PSUM)."""
                for sqc in range(b * (QB // 128), (b + 1) * (QB // 128)):
                    for eb in range(2):
                        ps = gps.tile([128, 512], f32, tag="gp", name="fc_ps")
                        nc.tensor.matmul(
                            ps[:], o_f16[:, 0, sqc * 128:(sqc + 1) * 128],
                            wo_t[:, 0, eb * 512:(eb + 1) * 512],
                            start=True, stop=False)
                        nc.tensor.matmul(
                            ps[:], o_f16[:, 1, sqc * 128:(sqc + 1) * 128],
                            wo_t[:, 1, eb * 512:(eb + 1) * 512],
                            start=False, stop=True)
                        ob = opool.tile([128, 512], f16, tag="ob")
                        nc.vector.tensor_copy(out=ob[:], in_=ps[:])
                        nc.sync.dma_start(
                            out_d[sqc * 128:(sqc + 1) * 128,
                                  eb * 512:(eb + 1) * 512],
                            ob[:])

            # lead-in: kpT fully, first qpT block (vp projected inside step 0)
            proj_k()
            nc.sync.dma_start(wq_t[:], wqT.rearrange("(ko p) m -> p ko m", p=128))
            proj_q(0)
            nc.sync.dma_start(wv_t[:], wvT.rearrange("(ko p) m -> p ko m", p=128))
            nc.sync.dma_start(wo_t[:], woT.rearrange("(ko p) n -> p ko n", p=128))

            def av_chunk(state, psavs, sk0, sk1):
                """AV matmuls for skc in [sk0, sk1) of the previous block,
                one contiguous accumulation chain per head."""
                ppt, pqb, pet = state
                for j in range(2):
                    hl = 2 * ppt + j
                    for skc in range(sk0, sk1):
                        nc.tensor.matmul(
                            psavs[j][0:D + 1, :],
                            vp[:, skc, hl * (D + 1):(hl + 1) * (D + 1)],
                            pet[:, skc, j, :],
                            start=(skc == 0), stop=(skc == nsk - 1))

            # software-pipelined attention over 8 (pt, qb) steps
            state = None  # (pt, qb, et tile) awaiting AV
            GRP = 4
            for pt in range(2):
                for qb in range(NQB):
                    q0 = qb * QB
                    et_t = etpool.tile([128, nsk, 2, QB], bf16, tag="et",
                                       name="et_t")
                    psavs = None
                    if state is not None:
                        psavs = [avps.tile([128, QB], f32, tag="av",
                                           name=f"psav{j}") for j in range(2)]
                    for g0 in range(0, nsk, GRP):
                        g1 = min(g0 + GRP, nsk)
                        if state is not None:
                            av_chunk(state, psavs, g0, g1)
                        for skc in range(g0, g1):
                            psx = pxps.tile([128, 2, QB], f32, tag="px",
                                            name="psx")
                            for j in range(2):
                                nc.tensor.matmul(
                                    psx[:, j, :],
                                    kpT[64 * j:64 * j + 64, pt,
                                        skc * 128:(skc + 1) * 128],
                                    qpT[64 * j:64 * j + 64, pt, q0:q0 + QB],
                                    start=True, stop=True,
                                    tile_position=(64 * j, 0))
                            nc.scalar.activation(
                                et_t[:, skc, :, :], psx[:], Act.Exp)
                    if state is not None:
                        finish_block(state[0], state[1], psavs)
                    state = (pt, qb, et_t)
                    # PE filler during the ACT-bound exp phase
                    if pt == 0:
                        if qb + 1 < NQB:
                            proj_q(qb + 1)
                        if qb == 0:
                            # vp must be complete before block-0's AV, which
                            # is emitted at the start of step 1
                            for sc in range(nsk):
                                proj_v(sc)
                    elif qb >= 1:
                        fc_block(qb - 1)
            # drain: AV + normalize of the last block, then remaining fc
            psavs = [avps.tile([128, QB], f32, tag="av", name=f"psavd{j}")
                     for j in range(2)]
            av_chunk(state, psavs, 0, nsk)
            finish_block(state[0], state[1], psavs)
            fc_block(3)

    if split_waits:
        _split_excess_waits(nc)
    return nc


def _prep_inputs(q, k, v, mask, W_qkv, b_qkv, W_out, b_out):
    """Host-side shard/layout prep. Returns (skv, in_maps)."""
    q = np.asarray(q, dtype=np.float32)
    k = np.asarray(k, dtype=np.float32)
    v = np.asarray(v, dtype=np.float32)
    mask = np.asarray(mask)
    W_qkv = np.asarray(W_qkv, dtype=np.float32)
    b_qkv = np.asarray(b_qkv, dtype=np.float32)
    W_out = np.asarray(W_out, dtype=np.float32)

    valid = [np.nonzero(mask[b, 0, 0] != 0)[0] for b in range(B)]
    cnts = [len(vi) for vi in valid]
    skv = max(128, max((c + 127) // 128 * 128 for c in cnts))

    # per-batch tensors
    qT, kTc, vTc, vms = [], [], [], []
    for b in range(B):
        qT.append(np.ascontiguousarray(q[b].T).astype(np.float16))
        kt = np.zeros((E, skv), np.float16)
        vt = np.zeros((E, skv), np.float16)
        kt[:, :cnts[b]] = k[b][valid[b]].T
        vt[:, :cnts[b]] = v[b][valid[b]].T
        kTc.append(kt)
        vTc.append(vt)
        vm = np.zeros((skv,), np.float32)
        vm[:cnts[b]] = 1.0
        vms.append(vm)

    sel2 = np.zeros((2, 128), np.float32)
    sel2[0, :64] = 1.0
    sel2[1, 64:] = 1.0

    in_maps = []
    for c in range(NCORES):
        b, g = divmod(c, GROUPS)
        sl = slice(g * DC, (g + 1) * DC)
        in_maps.append({
            "xqT": qT[b], "xkT": kTc[b], "xvT": vTc[b],
            "wqT": np.ascontiguousarray(W_qkv[sl, :].T).astype(np.float16),
            "wkT": np.ascontiguousarray(W_qkv[E:][sl, :].T).astype(np.float16),
            "wvT": np.ascontiguousarray(W_qkv[2 * E:][sl, :].T).astype(np.float16),
            "woT": np.ascontiguousarray(W_out[:, sl].T).astype(np.float16),
            "bq": np.ascontiguousarray(b_qkv[sl]),
            "bk": np.ascontiguousarray(b_qkv[E:][sl]),
            "bv": np.ascontiguousarray(b_qkv[2 * E:][sl]),
            "vmask": vms[b],
            "sel2": sel2,
        })
    return skv, in_maps


def kernel(q, k, v, mask, W_qkv, b_qkv, W_out, b_out):
    from concourse import bass_utils

    skv, in_maps = _prep_inputs(q, k, v, mask, W_qkv, b_qkv, W_out, b_out)
    if skv not in _CACHE:
        _CACHE[skv] = _build(skv)
    nc = _CACHE[skv]

    trace = os.environ.get("KERNEL_TRACE") == "1"
    if trace:
        bass_utils.upload_artifacts = lambda tmpdir: "local://" + tmpdir
    res = bass_utils.run_bass_kernel_spmd(
        nc, in_maps, list(range(NCORES)), trace=trace)
    if trace:
        print(f"HW exec time: {res.exec_time_ns} ns")

    b_out = np.asarray(b_out, dtype=np.float32)
    out = np.zeros((B, S, E), np.float32)
    for c in range(NCORES):
        out[c // GROUPS] += res.results[c]["out"].astype(np.float32)
    out += b_out[None, None, :]
    return out


# revision 17
# speedup vs baseline: 1.6249x; 1.1375x over previous
"""Multi-head attention (B=2, S=2048, E=1024, H=16) on 8 TRN2 NeuronCores.

Sharding: batch x head-group. Core c handles batch b=c//4 and head group
g=c%4 (4 heads = 256 of E). Each core computes its heads' attention output
slice and a partial fc_out product [S, E]; the host sums the 4 partials per
batch and adds b_out.

Device-side math per core (all matmuls in float32r, full PE rate):
  qpT = (Wq_g @ q[b].T + bq)      [256, S]   (T layout: dims on partitions)
  kpT = (Wk_g @ k_c[b].T + bk)    [256, SKV] (k compressed by mask, padded)
  vp  = (v_c[b] @ Wv_g.T + bv)*m  [SKV, 4*65] (per head: 64 dims + ones col)
  S_T = kpT_h.T-chunks @ qpT_h    [SKV, S] per head (2 heads row-packed, K=64)
  E_T = exp(S_T)                  (no max-subtraction: |energy| <~ 60, safe)
  AV  = vp_aug.T @ E_T  -> [65, S]: rows 0-63 = unnormalized O_T, row 64 =
        softmax denominator (ones-column trick; pad rows contribute 0)
  O_T = AV[0:64] / AV[64]         (recip + PE partition-broadcast)
  out_partial = O_T.T @ Wo_g.T    [S, E] (both head-pairs accumulated in PSUM)

Mask handling is exact: masked K/V rows are removed on the host (gather),
so softmax(where(mask==0, -1e20, e)) == exp(e_valid)/sum(exp(e_valid)).

Pipeline: the attention loop is software-pipelined over 512-wide query
blocks: block s emits [AV of block s-1 | scores of block s | exp of block s]
interleaved per skv-chunk, so the Scalar engine (exp, the per-block
bottleneck) always has scores available and the PE back-fills with AV,
Q-projection (pt=0) or fc_out (pt=1) work. exp runs as one 1024-wide
instruction per skv-chunk over both row-packed heads (2 adjacent PSUM
banks) to amortize ACT fixed overheads. Softmax normalization is per-block
so fc_out can start before the whole pair finishes.
"""

import os

import numpy as np

B, S, E, H = 2, 2048, 1024, 16
D = E // H           # 64
NCORES = 8
GROUPS = 4           # head groups per batch (cores per batch)
HPG = H // GROUPS    # 4 heads per core
DC = E // GROUPS     # 256 dims per core
NB = E // 128        # 8 contraction chunks over E
QB = 512             # query block width
NQB = S // QB        # 4

_CACHE = {}


def _split_excess_waits(nc, max_waits=1):
    """walrus rejects instructions carrying >1 sem wait; spread extras onto
    single-wait NoOps inserted before the instruction on the same engine."""
    import concourse.mybir as mybir

    n_split = 0
    for f in nc.m.functions:
        for bb in f.blocks:
            out, changed = [], False
            for ins in bb.instructions:
                si = ins.sync_info
                if si is not None and si.on_wait is not None and len(si.on_wait) > max_waits:
                    waits = list(si.on_wait)
                    for w in waits[:-max_waits]:
                        out.append(mybir.InstNoOp(
                            name=nc.get_next_instruction_name(),
                            engine=ins.engine, ins=[], outs=[],
                            sync_info=mybir.SyncInfo(on_wait=[w], on_update=[])))
                        n_split += 1
                    ins.sync_info = mybir.SyncInfo(
                        on_wait=waits[-max_waits:], on_update=list(si.on_update))
                    changed = True
                out.append(ins)
            if changed:
                bb.instructions = out
    return n_split


def _build(skv, split_waits=True):
    import concourse.bass as bass
    import concourse.mybir as mybir
    import concourse.tile as tile

    f32 = mybir.dt.float32
    f32r = mybir.dt.float32r
    f16 = mybir.dt.float16
    bf16 = mybir.dt.bfloat16
    Alu = mybir.AluOpType
    Act = mybir.ActivationFunctionType

    nsk = skv // 128
    kblocks = []
    rem = skv
    while rem > 0:
        w = 384 if rem % 384 == 0 else min(256, rem)
        kblocks.append(w)
        rem -= w

    nc = bass.Bass()
    xqT = nc.declare_dram_parameter("xqT", [E, S], f16, isOutput=False)
    xkT = nc.declare_dram_parameter("xkT", [E, skv], f16, isOutput=False)
    xvT = nc.declare_dram_parameter("xvT", [E, skv], f16, isOutput=False)
    wqT = nc.declare_dram_parameter("wqT", [E, DC], f16, isOutput=False)
    wkT = nc.declare_dram_parameter("wkT", [E, DC], f16, isOutput=False)
    wvT = nc.declare_dram_parameter("wvT", [E, DC], f16, isOutput=False)
    woT = nc.declare_dram_parameter("woT", [DC, E], f16, isOutput=False)
    bq_d = nc.declare_dram_parameter("bq", [DC], f32, isOutput=False)
    bk_d = nc.declare_dram_parameter("bk", [DC], f32, isOutput=False)
    bv_d = nc.declare_dram_parameter("bv", [DC], f32, isOutput=False)
    vm_d = nc.declare_dram_parameter("vmask", [skv], f32, isOutput=False)
    sel2_d = nc.declare_dram_parameter("sel2", [2, 128], f32r, isOutput=False)
    out_d = nc.declare_dram_parameter("out", [S, E], f16, isOutput=True)

    xqT_r = xqT.rearrange("(ko p) s -> p ko s", p=128)
    xkT_r = xkT.rearrange("(ko p) s -> p ko s", p=128)
    xvT_r = xvT.rearrange("(ko p) s -> p ko s", p=128)

    with tile.TileContext(nc) as tc:
        with (
            tc.tile_pool(name="weights", bufs=1) as wpool,
            tc.tile_pool(name="consts", bufs=1) as cpool,
            tc.tile_pool(name="persist", bufs=1) as ppool,
            tc.tile_pool(name="small", bufs=2) as smpool,
            tc.tile_pool(name="stream", bufs=1) as spool,
            tc.tile_pool(name="px_ps", bufs=2, space="PSUM") as pxps,
            tc.tile_pool(name="av_ps", bufs=2, space="PSUM") as avps,
            tc.tile_pool(name="gen_ps", bufs=2, space="PSUM") as gps,
            tc.tile_pool(name="et", bufs=2) as etpool,
            tc.tile_pool(name="outp", bufs=3) as opool,
            tc.tile_pool(name="rcr", bufs=2) as rcpool,
        ):
            # ---- tiles ----
            wk_t = wpool.tile([128, NB, DC], f16, tag="wk", name="wk_t")
            wq_t = wpool.tile([128, NB, DC], f16, tag="wq", name="wq_t")
            wv_t = wpool.tile([128, NB, DC], f16, tag="wv", name="wv_t")
            wo_t = wpool.tile([128, DC // 128, E], f16, tag="wo", name="wo_t")
            bq_t = cpool.tile([128, 2], f32, tag="bq")
            bk_t = cpool.tile([128, 2], f32, tag="bk")
            bv_t = cpool.tile([128, DC], f32, tag="bv")
            vm_t = cpool.tile([128, nsk], f32, tag="vm")
            sel2_t = cpool.tile([2, 128], f32r, tag="sel2")
            xks = [spool.tile([128, NB, w], f16, tag=f"xk{i}", name="xk")
                   for i, w in enumerate(kblocks)]
            xqs = [spool.tile([128, NB, 512], f16, tag=f"xq{i}", name="xq")
                   for i in range(NQB)]
            xvs = [spool.tile([128, NB, 128], f16, tag=f"xv{i}", name="xv")
                  for i in range(nsk)]

            qpT = ppool.tile([128, 2, S], f16, tag="qpT")
            kpT = ppool.tile([128, 2, skv], f16, tag="kpT")
            vp = ppool.tile([128, nsk, HPG * (D + 1)], bf16, tag="vp")
            # per-j unnormalized O_T (rows 0-63) + denominator (row 64)
            o_un0 = ppool.tile([65, 2, S], f32, tag="o_un0")
            o_un1 = ppool.tile([65, 2, S], f32, tag="o_un1")
            o_uns = [o_un0, o_un1]
            o_f16 = ppool.tile([128, 2, S], f16, tag="o_f16")

            # ---- all input DMAs issued upfront in priority order ----
            nc.sync.dma_start(wk_t[:], wkT.rearrange("(ko p) m -> p ko m", p=128))
            nc.sync.dma_start(bk_t[:], bk_d.rearrange("(c p) -> p c", p=128))
            nc.sync.dma_start(bq_t[:], bq_d.rearrange("(c p) -> p c", p=128))
            nc.sync.dma_start(bv_t[:], bv_d[None, :].to_broadcast((128, DC)))
            nc.sync.dma_start(vm_t[:], vm_d.rearrange("(s p) -> p s", p=128))
            nc.sync.dma_start(sel2_t[:], sel2_d[:])
            off = 0
            for i, w in enumerate(kblocks):
                nc.sync.dma_start(xks[i][:], xkT_r[:, :, off:off + w])
                off += w
                if i == 0:
                    nc.sync.dma_start(
                        wq_t[:], wqT.rearrange("(ko p) m -> p ko m", p=128))
                    nc.sync.dma_start(xqs[0][:], xqT_r[:, :, 0:512])
            nc.sync.dma_start(wv_t[:], wvT.rearrange("(ko p) m -> p ko m", p=128))
            nc.sync.dma_start(xqs[1][:], xqT_r[:, :, 512:1024])
            for sc in range(nsk):
                nc.sync.dma_start(xvs[sc][:], xvT_r[:, :, sc * 128:(sc + 1) * 128])
            for nb in range(2, NQB):
                nc.sync.dma_start(xqs[nb][:], xqT_r[:, :, nb * 512:(nb + 1) * 512])
            nc.sync.dma_start(wo_t[:], woT.rearrange("(ko p) n -> p ko n", p=128))

            def mm_k():
                off = 0
                for i, w in enumerate(kblocks):
                    for mc in range(2):
                        ps = gps.tile([128, 512], f32, tag="gp", name="kp_ps")[:, :w]
                        for kc in range(NB):
                            nc.tensor.matmul(
                                ps[:], wk_t[:, kc, mc * 128:(mc + 1) * 128],
                                xks[i][:, kc, :], start=(kc == 0), stop=(kc == NB - 1))
                        nc.vector.tensor_tensor(
                            out=kpT[:, mc, off:off + w], in0=ps[:],
                            in1=bk_t[:, mc:mc + 1].to_broadcast((128, w)), op=Alu.add)
                    off += w

            def mm_q(nb):
                for mc in range(2):
                    ps = gps.tile([128, 512], f32, tag="gp", name="qp_ps")
                    for kc in range(NB):
                        nc.tensor.matmul(
                            ps[:], wq_t[:, kc, mc * 128:(mc + 1) * 128],
                            xqs[nb][:, kc, :], start=(kc == 0), stop=(kc == NB - 1))
                    nc.vector.tensor_tensor(
                        out=qpT[:, mc, nb * 512:(nb + 1) * 512], in0=ps[:],
                        in1=bq_t[:, mc:mc + 1].to_broadcast((128, 512)), op=Alu.add)

            def mm_v(sc):
                ps = gps.tile([128, 512], f32, tag="gp", name="vp_ps")[:, :DC]
                for kc in range(NB):
                    nc.tensor.matmul(
                        ps[:], xvs[sc][:, kc, :], wv_t[:, kc, :],
                        start=(kc == 0), stop=(kc == NB - 1))
                t1 = smpool.tile([128, DC], f32, tag="vtmp")
                nc.vector.tensor_tensor(out=t1[:], in0=ps[:], in1=bv_t[:], op=Alu.add)
                vps = vp[:, sc, :].rearrange("p (h w) -> p h w", w=D + 1)
                nc.vector.tensor_tensor(
                    out=vps[:, :, 0:D],
                    in0=t1.rearrange("p (h w) -> p h w", w=D),
                    in1=vm_t[:, sc:sc + 1, None].to_broadcast((128, HPG, D)),
                    op=Alu.mult)
                nc.vector.tensor_copy(
                    out=vps[:, :, D:D + 1],
                    in_=vm_t[:, sc:sc + 1, None].to_broadcast((128, HPG, 1)))

            def finish_block(pt, qb, psavs):
                """AV copy + per-block softmax normalize for query block qb
                of pair pt (runs one pipeline step after its AV matmuls)."""
                q0 = qb * QB
                for j in range(2):
                    nc.vector.tensor_copy(
                        out=o_uns[j][0:D + 1, pt, q0:q0 + QB],
                        in_=psavs[j][0:D + 1, :])
                rc_r = rcpool.tile([2, QB], f32r, tag="rcr")
                for j in range(2):
                    s128 = smpool.tile([128, QB // 128], f32, tag="s128")
                    nc.gpsimd.dma_start(s128[:], o_uns[j][D:D + 1, pt, q0:q0 + QB])
                    nc.vector.reciprocal(out=s128[:], in_=s128[:])
                    r128 = smpool.tile([128, QB // 128], f32r, tag="r128")
                    nc.vector.tensor_copy(out=r128[:], in_=s128[:])
                    nc.gpsimd.dma_start(rc_r[j:j + 1, :], r128[:])
                # one PE broadcast for both heads: sel2 routes row j of rc_r
                # to partitions 64j..64j+63
                rc_ps = gps.tile([128, 512], f32, tag="gp", name="rc_ps")
                nc.tensor.matmul(
                    rc_ps[:], sel2_t[:], rc_r[:], start=True, stop=True)
                for j in range(2):
                    nc.vector.tensor_tensor(
                        out=o_f16[64 * j:64 * j + 64, pt, q0:q0 + QB],
                        in0=o_uns[j][0:D, pt, q0:q0 + QB],
                        in1=rc_ps[64 * j:64 * j + 64, :], op=Alu.mult)

            def fc_block(b):
                """fc_out for query block b (both pairs accumulated in PSUM)."""
                for sqc in range(b * (QB // 128), (b + 1) * (QB // 128)):
                    ob = opool.tile([128, 2, 512], f16, tag="ob")
                    for eb in range(2):
                        ps = gps.tile([128, 512], f32, tag="gp", name="fc_ps")
                        nc.tensor.matmul(
                            ps[:], o_f16[:, 0, sqc * 128:(sqc + 1) * 128],
                            wo_t[:, 0, eb * 512:(eb + 1) * 512],
                            start=True, stop=False)
                        nc.tensor.matmul(
                            ps[:], o_f16[:, 1, sqc * 128:(sqc + 1) * 128],
                            wo_t[:, 1, eb * 512:(eb + 1) * 512],
                            start=False, stop=True)
                        nc.vector.tensor_copy(out=ob[:, eb, :], in_=ps[:])
                    nc.sync.dma_start(
                        out_d[sqc * 128:(sqc + 1) * 128, :], ob[:])

            # lead-in: kpT fully, first qpT block (vp projected inside step 0)
            mm_k()
            mm_q(0)

            def av_chunk(state, psavs, sk0, sk1):
                """AV matmuls for skc in [sk0, sk1) of the previous block,
                one contiguous accumulation chain per head."""
                ppt, pqb, pet = state
                for j in range(2):
                    hl = 2 * ppt + j
                    for skc in range(sk0, sk1):
                        nc.tensor.matmul(
                            psavs[j][0:D + 1, :],
                            vp[:, skc, hl * (D + 1):(hl + 1) * (D + 1)],
                            pet[:, skc, j, :],
                            start=(skc == 0), stop=(skc == nsk - 1))

            # software-pipelined attention over 8 (pt, qb) steps
            state = None  # (pt, qb, et tile) awaiting AV
            GRP = 4
            for pt in range(2):
                for qb in range(NQB):
                    q0 = qb * QB
                    et_t = etpool.tile([128, nsk, 2, QB], bf16, tag="et",
                                       name="et_t")
                    psavs = None
                    if state is not None:
                        psavs = [avps.tile([128, QB], f32, tag="av",
                                           name=f"psav{j}") for j in range(2)]
                    for g0 in range(0, nsk, GRP):
                        g1 = min(g0 + GRP, nsk)
                        if state is not None:
                            av_chunk(state, psavs, g0, g1)
                        for skc in range(g0, g1):
                            psx = pxps.tile([128, 2, QB], f32, tag="px",
                                            name="psx")
                            for j in range(2):
                                nc.tensor.matmul(
                                    psx[:, j, :],
                                    kpT[64 * j:64 * j + 64, pt,
                                        skc * 128:(skc + 1) * 128],
                                    qpT[64 * j:64 * j + 64, pt, q0:q0 + QB],
                                    start=True, stop=True,
                                    tile_position=(64 * j, 0))
                            nc.scalar.activation(
                                et_t[:, skc, :, :], psx[:], Act.Exp)
                    # PE filler during the ACT-bound exp phase. In pt=0 the
                    # filler is emitted before finish_block so its matmuls
                    # outrank the normalize chain; in pt=1 fc needs the
                    # normalize, so finish goes first.
                    if pt == 0:
                        if qb + 1 < NQB:
                            mm_q(qb + 1)
                        if qb == 0:
                            # vp must be complete before block-0's AV, which
                            # is emitted at the start of step 1
                            for sc in range(nsk):
                                mm_v(sc)
                        if state is not None:
                            finish_block(state[0], state[1], psavs)
                    else:
                        if state is not None:
                            finish_block(state[0], state[1], psavs)
                        if qb >= 1:
                            fc_block(qb - 1)
                    state = (pt, qb, et_t)
            # drain: AV + normalize of the last block, then remaining fc
            psavs = [avps.tile([128, QB], f32, tag="av", name=f"psavd{j}")
                     for j in range(2)]
            av_chunk(state, psavs, 0, nsk)
            finish_block(state[0], state[1], psavs)
            fc_block(3)

    if split_waits:
        _split_excess_waits(nc)
    return nc


def _prep_inputs(q, k, v, mask, W_qkv, b_qkv, W_out, b_out):
    """Host-side shard/layout prep. Returns (skv, in_maps)."""
    q = np.asarray(q, dtype=np.float32)
    k = np.asarray(k, dtype=np.float32)
    v = np.asarray(v, dtype=np.float32)
    mask = np.asarray(mask)
    W_qkv = np.asarray(W_qkv, dtype=np.float32)
    b_qkv = np.asarray(b_qkv, dtype=np.float32)
    W_out = np.asarray(W_out, dtype=np.float32)

    valid = [np.nonzero(mask[b, 0, 0] != 0)[0] for b in range(B)]
    cnts = [len(vi) for vi in valid]
    skv = max(128, max((c + 127) // 128 * 128 for c in cnts))

    # per-batch tensors
    qT, kTc, vTc, vms = [], [], [], []
    for b in range(B):
        qT.append(np.ascontiguousarray(q[b].T).astype(np.float16))
        kt = np.zeros((E, skv), np.float16)
        vt = np.zeros((E, skv), np.float16)
        kt[:, :cnts[b]] = k[b][valid[b]].T
        vt[:, :cnts[b]] = v[b][valid[b]].T
        kTc.append(kt)
        vTc.append(vt)
        vm = np.zeros((skv,), np.float32)
        vm[:cnts[b]] = 1.0
        vms.append(vm)

    sel2 = np.zeros((2, 128), np.float32)
    sel2[0, :64] = 1.0
    sel2[1, 64:] = 1.0

    in_maps = []
    for c in range(NCORES):
        b, g = divmod(c, GROUPS)
        sl = slice(g * DC, (g + 1) * DC)
        in_maps.append({
            "xqT": qT[b], "xkT": kTc[b], "xvT": vTc[b],
            "wqT": np.ascontiguousarray(W_qkv[sl, :].T).astype(np.float16),
            "wkT": np.ascontiguousarray(W_qkv[E:][sl, :].T).astype(np.float16),
            "wvT": np.ascontiguousarray(W_qkv[2 * E:][sl, :].T).astype(np.float16),
            "woT": np.ascontiguousarray(W_out[:, sl].T).astype(np.float16),
            "bq": np.ascontiguousarray(b_qkv[sl]),
            "bk": np.ascontiguousarray(b_qkv[E:][sl]),
            "bv": np.ascontiguousarray(b_qkv[2 * E:][sl]),
            "vmask": vms[b],
            "sel2": sel2,
        })
    return skv, in_maps


def kernel(q, k, v, mask, W_qkv, b_qkv, W_out, b_out):
    from concourse import bass_utils

    skv, in_maps = _prep_inputs(q, k, v, mask, W_qkv, b_qkv, W_out, b_out)
    if skv not in _CACHE:
        _CACHE[skv] = _build(skv)
    nc = _CACHE[skv]

    trace = os.environ.get("KERNEL_TRACE") == "1"
    if trace:
        bass_utils.upload_artifacts = lambda tmpdir: "local://" + tmpdir
    res = bass_utils.run_bass_kernel_spmd(
        nc, in_maps, list(range(NCORES)), trace=trace)
    if trace:
        print(f"HW exec time: {res.exec_time_ns} ns")

    b_out = np.asarray(b_out, dtype=np.float32)
    out = np.zeros((B, S, E), np.float32)
    for c in range(NCORES):
        out[c // GROUPS] += res.results[c]["out"].astype(np.float32)
    out += b_out[None, None, :]
    return out
